# revision 37
# baseline (speedup 1.0000x reference)
"""Trainium2 Bass kernel for the ACG mixture log-likelihood.

Reference computation (N=100000, P=256, K=64, R=16):
    D_k   = I_R + M_k^T M_k
    quad  = einsum('np,kpr,krs,kqs,nq->kn', X, M, inv(D), M, X)
    dens  = logSA - 0.5*logdet(D_k) - (P/2)*log(1 - quad) + log_softmax(pi)_k
    out   = sum_n logsumexp_k dens

Algebraic folding done on the host (tiny, O(K*P*R)):
    inv(D_k) = C_k^{-T} C_k^{-1}   (Cholesky)  =>  quad_kn = ||X W_k||^2
    with W_k = M_k C_k^{-T}.  Per-component constants fold into a column
    scale sqrt(g_k), g_k = exp((C0 - c_k)/128), so on the device:
        y     = X_bf16 @ W_bf16                  (one [*,256]x[256,1024] matmul)
        q_k   = sum_r y_{r,k}^2                  (squares + add-tree)
        term  = (g_k - q_k)^{-128} = exp(-128*ln(g_k - q_k))
        out  += ln(sum_k term)                   (+ N*C0 host-side)

Sharding: data-parallel over N across 8 cores (X rows pre-transposed and
bf16-cast on the host as part of the shard layout); W / g replicated.  Each
core returns per-(partition, tile) partial sums ln(sum_k term); the host
gathers the 8 small partials and finishes the scalar reduction in float64.

Self-contained: only needs /opt/trn_rl_repo (the in-container Bass repo).
"""

import math
import os
import sys

import numpy as np

sys.path.insert(0, "/opt/trn_rl_repo")

import ml_dtypes

BF16 = ml_dtypes.bfloat16

# Problem constants (hardcoded per task instructions).
N_TOT, P, K, R = 100000, 256, 64, 16
KR = K * R                     # 1024 matmul output columns
NCORES = 8
NSH_REAL = N_TOT // NCORES     # 12500 samples per core
TILE = 128                     # samples per matmul tile (PSUM partition dim)
NT = (NSH_REAL + TILE - 1) // TILE   # 98 tiles
NSH = NT * TILE                # 12544 padded samples per core
NPAIR = NT // 2                # 49 pairs of tiles (one PSUM tile per pair)
CHUNK_PAIRS = 7                # X chunks resident in SBUF: 7 pairs each
NCHUNK = NPAIR // CHUNK_PAIRS  # 7 chunks
CHUNK_S = CHUNK_PAIRS * 2 * TILE     # 1792 samples per chunk
BATCH_PAIRS = 7                # pairs per logsumexp batch (14 tiles)

_STATE: dict = {}

LAST_EXEC_NS = None


def _fold_params(M: np.ndarray, pi: np.ndarray):
    """Host-side folding of the tiny per-component parameters (float64)."""
    M64 = M.astype(np.float64)                  # [K, P, R]
    pi64 = pi.astype(np.float64)
    D = np.eye(R)[None, :, :] + np.einsum("kpr,kps->krs", M64, M64)
    C = np.linalg.cholesky(D)                   # D = C C^T
    logdet = 2.0 * np.log(np.diagonal(C, axis1=1, axis2=2)).sum(axis=1)
    # W_k = M_k C_k^{-T}  =>  ||x W_k||^2 = x^T M D^{-1} M^T x
    Cinv = np.linalg.inv(C)                     # [K, R, R]
    W = np.einsum("kpr,ksr->kps", M64, Cinv)    # M_k @ C_k^{-T}
    mx = pi64.max()
    logpi = pi64 - (mx + math.log(np.exp(pi64 - mx).sum()))
    half_p = 0.5 * P
    logSA = math.lgamma(half_p) - math.log(2.0) - half_p * math.log(math.pi)
    c = logSA - 0.5 * logdet + logpi            # [K]
    C0 = float(c.max())
    g = np.exp((C0 - c) / 128.0)                # [K], >= 1
    Ws = W * np.sqrt(g)[:, None, None]          # fold scale into matmul weights
    # Column layout r-major: f = r*K + k, so the add-tree halves are contiguous.
    Wf = np.transpose(Ws, (1, 2, 0)).reshape(P, KR)   # [P, (r,k)]
    lse_c = C0 + math.log(np.exp(c - C0).sum())
    return Wf, g, C0, lse_c


def _build():
    import concourse.mybir as mybir
    import concourse.tile as tile
    from concourse import bacc

    f32 = mybir.dt.float32
    bf16 = mybir.dt.bfloat16
    AF = mybir.ActivationFunctionType
    ALU = mybir.AluOpType
    AX = mybir.AxisListType

    # All three transcendental-ish funcs we use (Square, Ln, Exp) live in the
    # "natural_log_exp_and_others" ACT table set, but the table-load inserter
    # may pick a different set per function and thrash (measured 28 loads,
    # 43us).  Strip those funcs from every other set so one load suffices.
    if not _STATE.get("act_tables_patched"):
        _orig_tables = bacc.get_activation_tables

        def _patched_tables(arch):
            tabs = _orig_tables(arch)
            keep = "natural_log_exp_and_others"
            if keep in tabs:
                for name, fns in tabs.items():
                    if name != keep:
                        fns.difference_update({AF.Square, AF.Ln, AF.Exp})
            return tabs

        bacc.get_activation_tables = _patched_tables
        _STATE["act_tables_patched"] = True

    nc = bacc.Bacc("TRN2", target_bir_lowering=False, debug=False,
                   num_devices=NCORES)
    xt_d = nc.dram_tensor("xt", [2, 128, NSH], bf16, kind="ExternalInput")
    w_d = nc.dram_tensor("w", [2, 128, KR], bf16, kind="ExternalInput")
    g_d = nc.dram_tensor("g", [128, 2 * BATCH_PAIRS, K], f32, kind="ExternalInput")
    out_d = nc.dram_tensor("out", [128, NT], f32, kind="ExternalOutput")

    with tile.TileContext(nc) as tc:
        with (
            tc.tile_pool(name="const", bufs=1) as cpool,
            tc.tile_pool(name="psum", bufs=2, space="PSUM") as ppool,
            tc.tile_pool(name="ysq", bufs=6) as ypool,
            tc.tile_pool(name="tree", bufs=6) as tpool,
            tc.tile_pool(name="batch", bufs=3) as bpool,
            tc.tile_pool(name="dram", bufs=1, space="DRAM") as dpool,
        ):
            w_sb = cpool.tile([128, 2, KR], bf16, tag="w")
            nc.sync.dma_start(w_sb[:, 0, 0:512], w_d.ap()[0, :, 0:512])
            nc.sync.dma_start(w_sb[:, 0, 512:KR], w_d.ap()[0, :, 512:KR])
            sredb = cpool.tile([128, NT], f32, tag="sredb")
            xt_chunks = []
            for ci in range(NCHUNK):
                xc = cpool.tile([128, 2, CHUNK_S], bf16, tag=f"xt{ci}")
                nsub = 7 if ci == 0 else 1
                sub = CHUNK_S // nsub
                for j in range(nsub):
                    for po in range(2):
                        nc.sync.dma_start(
                            xc[:, po, j * sub:(j + 1) * sub],
                            xt_d.ap()[po, :, ci * CHUNK_S + j * sub:
                                      ci * CHUNK_S + (j + 1) * sub],
                        )
                    if ci == 0 and j == 0:
                        # second half of W right after pair 0's data so the
                        # first matmuls start as early as possible
                        nc.sync.dma_start(w_sb[:, 1, 0:512],
                                          w_d.ap()[1, :, 0:512])
                        nc.sync.dma_start(w_sb[:, 1, 512:KR],
                                          w_d.ap()[1, :, 512:KR])
                xt_chunks.append(xc)
            gb_sb = cpool.tile([128, 2 * BATCH_PAIRS, K], f32, tag="g")
            nc.sync.dma_start(gb_sb[:], g_d.ap())

            sizes = [BATCH_PAIRS] * 6 + [BATCH_PAIRS - 1, 1]
            assert sum(sizes) == NPAIR
            starts = [sum(sizes[:i]) for i in range(len(sizes))]
            for b, (st, sz) in enumerate(zip(starts, sizes)):
                pairs = range(st, st + sz)
                bs = 2 * len(pairs)           # sample-tiles in this batch
                qdb = bpool.tile([128, 2 * BATCH_PAIRS, K], bf16, tag="qdb")
                for pi_, pr in enumerate(pairs):
                    ci, pio = divmod(pr, CHUNK_PAIRS)
                    xc = xt_chunks[ci]
                    s0 = pio * 2 * TILE
                    py = ppool.tile([128, 2, KR], f32, tag="py")
                    for po in range(2):
                        for t in range(2):
                            lhsT = xc[:, po, s0 + t * TILE: s0 + (t + 1) * TILE]
                            for nh in range(2):
                                nc.tensor.matmul(
                                    py[:, t, nh * 512:(nh + 1) * 512],
                                    lhsT,
                                    w_sb[:, po, nh * 512:(nh + 1) * 512],
                                    start=(po == 0),
                                    stop=(po == 1),
                                )
                    yflat = py.rearrange("p t f -> p (t f)")
                    ysq = ypool.tile([128, 2, KR], bf16, tag="ysq")
                    ysqf = ysq.rearrange("p t f -> p (t f)")
                    nc.scalar.activation(ysqf[:], yflat[:], AF.Square)
                    # Two independent per-tile add-tree chains (flat 2D
                    # contiguous bf16 ops run in the DVE 2x mode); the final
                    # small add runs on the otherwise-idle GpSimd engine.
                    for t in range(2):
                        yt = ysq[:, t]
                        t1 = tpool.tile([128, 512], bf16, tag="t1")
                        nc.vector.tensor_add(t1[:], yt[:, 0:512], yt[:, 512:1024])
                        t2 = tpool.tile([128, 256], bf16, tag="t2")
                        nc.vector.tensor_add(t2[:], t1[:, 0:256], t1[:, 256:512])
                        t3 = tpool.tile([128, 128], bf16, tag="t3")
                        nc.vector.tensor_add(t3[:], t2[:, 0:128], t2[:, 128:256])
                        nc.gpsimd.tensor_tensor(qdb[:, 2 * pi_ + t, :],
                                                t3[:, 0:64], t3[:, 64:128],
                                                ALU.add)
                # pdf_hat = g_k - g_k*quad, batched (strictly positive here)
                pdfb = bpool.tile([128, 2 * BATCH_PAIRS, K], bf16, tag="pdfb")
                nc.vector.tensor_tensor(pdfb[:, :bs], gb_sb[:, :bs], qdb[:, :bs],
                                        ALU.subtract)
                lp = bpool.tile([128, 2 * BATCH_PAIRS, K], f32, tag="lp")
                nc.scalar.activation(lp[:, :bs], pdfb[:, :bs], AF.Ln)
                ee = bpool.tile([128, 2 * BATCH_PAIRS, K], f32, tag="ee")
                nc.scalar.activation(ee[:, :bs], lp[:, :bs], AF.Exp, scale=-128.0)
                nc.vector.tensor_reduce(sredb[:, 2 * st:2 * st + bs],
                                        ee[:, :bs], axis=AX.X, op=ALU.add)

            lsall = cpool.tile([128, NT], f32, tag="lsall")
            nc.scalar.activation(lsall[:], sredb[:], AF.Ln)
            nc.sync.dma_start(out_d.ap(), lsall[:])

    nc.compile()
    return nc


def _maybe_register_trace_hook():
    """Register the axon NTFF profile hook if the image lacks antenv.axon_hooks."""
    try:
        from antenv.axon_hooks import get_axon_ntff_profile_hook  # noqa: F401
        return
    except ImportError:
        pass
    import contextlib
    import ctypes
    import types

    so_path = "/opt/axon/libaxon_pjrt.so"
    if not os.path.exists(so_path):
        return
    lib = ctypes.CDLL(so_path)
    if not hasattr(lib, "axon_start_nrt_profile"):
        return
    lib.axon_start_nrt_profile.argtypes = [ctypes.POINTER(ctypes.c_int64),
                                           ctypes.c_size_t]
    lib.axon_start_nrt_profile.restype = ctypes.c_int64
    lib.axon_stop_nrt_profile.argtypes = [ctypes.c_char_p]
    lib.axon_stop_nrt_profile.restype = ctypes.c_int64

    @contextlib.contextmanager
    def _hook(output_dir, device_ids):
        import jax
        jax.devices()
        if device_ids:
            ids = (ctypes.c_int64 * len(device_ids))(*device_ids)
            rc = lib.axon_start_nrt_profile(ids, len(device_ids))
        else:
            rc = lib.axon_start_nrt_profile(None, 0)
        if rc != 0:
            raise RuntimeError(f"axon_start_nrt_profile rc={rc}")
        try:
            yield
        finally:
            n = lib.axon_stop_nrt_profile(str(output_dir).encode())
            print(f"ntff profile: {n} file(s) -> {output_dir}", file=sys.stderr)

    mod = types.ModuleType("antenv.axon_hooks")
    mod.get_axon_ntff_profile_hook = lambda: _hook
    mod.set_axon_ntff_profile_hook = lambda h: None
    sys.modules["antenv.axon_hooks"] = mod


def kernel(X: np.ndarray, M: np.ndarray, pi: np.ndarray) -> np.ndarray:
    global LAST_EXEC_NS
    from concourse.bass_utils import run_bass_kernel_spmd

    if "nc" not in _STATE:
        _STATE["nc"] = _build()
    nc = _STATE["nc"]

    Wf, g, C0, lse_c = _fold_params(M, pi)
    w_host = np.ascontiguousarray(Wf.astype(BF16).reshape(2, 128, KR))
    g_host = np.ascontiguousarray(
        np.broadcast_to(g.astype(np.float32)[None, None, :], (128, 2 * BATCH_PAIRS, K)))

    in_maps = []
    for cix in range(NCORES):
        xpad = np.zeros((NSH, P), dtype=BF16)
        xpad[:NSH_REAL] = X[cix * NSH_REAL:(cix + 1) * NSH_REAL].astype(BF16)
        xt = np.ascontiguousarray(xpad.T).reshape(2, 128, NSH)
        in_maps.append({"xt": xt, "w": w_host, "g": g_host})

    trace = bool(int(os.environ.get("KERNEL_TRACE", "0")))
    if trace:
        _maybe_register_trace_hook()
    res = run_bass_kernel_spmd(nc, in_maps, core_ids=list(range(NCORES)),
                               trace=trace)
    LAST_EXEC_NS = res.exec_time_ns
    if trace and res.exec_time_ns is not None:
        print(f"HW exec time: {res.exec_time_ns} ns")
        if res.instructions_and_trace is not None:
            print(f"trace: {res.instructions_and_trace[1]}")

    total_dev = float(sum(r["out"].astype(np.float64).sum()
                          for r in res.results))
    n_pad = NSH * NCORES - N_TOT
    ans = total_dev + N_TOT * C0 - n_pad * (lse_c - C0)
    return np.asarray(ans, dtype=np.float32)
